# revision 48
# baseline (speedup 1.0000x reference)
"""Trainium2 Bass kernel for nn_DigitCap (sparse_attention).

Math note: the reference's softmax is over a size-1 axis, so C == 1 exactly
and the N x N attention matrix A is dead code.  The computation collapses to

    S[b,d,i]  = sum_{n,j} (1 + B[d,n]) * W[d,n,i,j] * U[b,n,j]
    out[b,d,:] = (1 - exp(-|S|)) * S / (|S| + 1e-7)

Sharding: 2 batch halves x 4 digit groups {0,1,2},{3,4,5},{6,7,8},{9,-,-}
over the 8 cores (core k: batch half k//4, digit group k%4).  This is the
byte-optimal grid: per core 256KB of U + 384KB of W (bf16) vs 512+256 for
pure digit sharding -- the measured DMA fabric tops out ~190GB/s aggregate
(~95GB/s per queue), so bytes on the heaviest queue are the whole game.

Key implementation notes:
  * bf16 inputs (tolerance is 2e-2; bf16 keeps us ~2.5e-3): halves HBM
    traffic and runs the PE at 1 cycle/row instead of fp32's 4.
  * 6 input DMAs balanced over the SP/ACT/Pool queues, >=1KB per-partition
    runs, ordered so each piece's ~900ns completion-semaphore lands just
    before the PE needs it; each dma_start also costs ~600ns of sequencer.
  * single PSUM accumulation group across all 32 matmuls, consumption
    order [24-31, 16-23, 8-15, 0-7] tracking DMA arrival.
  * (1+B)*W fused scale runs per W-quarter on DVE so only a ~350ns scale
    gates the first PE burst.
  * epilogue needs sqrt AND exp, which live in different ACT tables
    (1283ns load each).  sqrt(x) = exp(0.5*ln(x)) lets the whole epilogue
    use the one natural_log+exp table, pre-warmed during the DMA phase;
    1/norm comes from DVE reciprocal(norm) in parallel with ACT's exp.
    ln(0) = -inf on the pad capsules -> NaN outputs there; host discards.
  * DVE does NOT interlock same-engine read-after-write (8-deep exec
    queue reads stale operands), so the finisher is two fused
    scalar_tensor_tensor ops with one self-semaphore between them:
        fac' = (et - 1) * rn ; out = (-S) * fac'
"""

import numpy as np
import ml_dtypes
from contextlib import ExitStack

import concourse.bass as bass
import concourse.mybir as mybir
from concourse.bass_utils import run_bass_kernel_spmd

F32 = mybir.dt.float32
BF16 = mybir.dt.bfloat16
AF = mybir.ActivationFunctionType
ALU = mybir.AluOpType
P = 128
D, DD, N, DP = 10, 512 // 64 + 8, 512, 8  # placeholder; fixed below
D, DD, N, DP = 10, 16, 512, 8     # digit caps, digit dim, primary caps, primary dim
K = N * DP                         # 4096 contraction
NCHUNK = K // P                    # 32 chunks of 128 contraction rows
NCORES = 8
BFULL = 64
GD = [[0, 1, 2], [3, 4, 5], [6, 7, 8], [9]]   # digit groups (t-slots)
DC = 3                             # d slots per core (pads where group < 3)
BLOC = 32                          # batch rows per core (half of 64)
DIC = DC * DD                      # 48 output cols per core
HC = NCHUNK // 2                   # 16
QC = NCHUNK // 4                   # 8 chunks per quarter
WHALF = HC * DIC                   # 768 cols per W half
BCOLS = NCHUNK * DC                # 96
UCOLS = NCHUNK * BLOC              # 1024
# wb DRAM/SBUF layout: [Wq3 | B | Wq2 | Wq1 | Wq0], consumption order,
# so the first DMA piece (Wq3+B, 120KB) lands ~1.2us before a half would
WQ = QC * DIC                      # 384 cols per W quarter
WB_COLS = 4 * WQ + BCOLS           # 1632
OFF_B = WQ                         # 384
OFF_WQ = {3: 0, 2: WQ + BCOLS, 1: 2 * WQ + BCOLS, 0: 3 * WQ + BCOLS}


def _wcol(c):
    """start col of chunk c's W block inside wb"""
    q = c // QC
    return OFF_WQ[q] + (c - q * QC) * DIC


def build_raw():
    nc = bass.Bass()
    u_t = nc.dram_tensor("u_t", [P, UCOLS], BF16, kind="ExternalInput")
    wb_t = nc.dram_tensor("wb_t", [P, WB_COLS], BF16, kind="ExternalInput")
    out = nc.dram_tensor("out", [BLOC, DIC], F32, kind="ExternalOutput")

    with ExitStack() as ctx:
        u_all = ctx.enter_context(nc.sbuf_tensor("u_all", [P, UCOLS], BF16))
        wb = ctx.enter_context(nc.sbuf_tensor("wb", [P, WB_COLS], BF16))
        ps = ctx.enter_context(nc.psum_tensor("ps", [BLOC, DIC], F32))
        sq = ctx.enter_context(nc.sbuf_tensor("sq", [BLOC, DIC], F32))
        ss = ctx.enter_context(nc.sbuf_tensor("ss", [BLOC, DC], F32))
        lss = ctx.enter_context(nc.sbuf_tensor("lss", [BLOC, DC], F32))
        nrm = ctx.enter_context(nc.sbuf_tensor("nrm", [BLOC, DC], F32))
        rn = ctx.enter_context(nc.sbuf_tensor("rn", [BLOC, DC], F32))
        et = ctx.enter_context(nc.sbuf_tensor("et", [BLOC, DC], F32))
        fac = ctx.enter_context(nc.sbuf_tensor("fac", [BLOC, DC], F32))
        ot = ctx.enter_context(nc.sbuf_tensor("ot", [BLOC, DIC], F32))
        warm = ctx.enter_context(nc.sbuf_tensor("warm", [1, 2], F32))
        sem_w = [ctx.enter_context(nc.semaphore(f"sem_w{q}")) for q in range(4)]
        # u piece sems, indexed by quarter q (chunks q*QC..(q+1)*QC)
        sem_u = [ctx.enter_context(nc.semaphore(f"sem_u{q}")) for q in range(4)]
        sem_sq = [ctx.enter_context(nc.semaphore(f"sem_sq{q}")) for q in range(4)]
        sem_pe = ctx.enter_context(nc.semaphore("sem_pe"))
        sem_nm = ctx.enter_context(nc.semaphore("sem_nm"))
        sem_rn = ctx.enter_context(nc.semaphore("sem_rn"))
        sem_act = ctx.enter_context(nc.semaphore("sem_act"))
        sem_fp = ctx.enter_context(nc.semaphore("sem_fp"))
        sem_fin = ctx.enter_context(nc.semaphore("sem_fin"))
        sem_out = ctx.enter_context(nc.semaphore("sem_out"))

        one = nc.const_aps.tensor(1.0, (1, 1), F32)

        def u_dma(eng, q, sem):
            eng.dma_start(
                u_all[:, q * QC * BLOC : (q + 1) * QC * BLOC],
                bass.AP(u_t, q * QC * BLOC, [[UCOLS, P], [1, QC * BLOC]]),
            ).then_inc(sem, 16)

        with nc.Block() as block:

            def w_dma(eng, q, ncols, sem):
                off = OFF_WQ[q]
                eng.dma_start(
                    wb[:, off : off + ncols],
                    bass.AP(wb_t, off, [[WB_COLS, P], [1, ncols]]),
                ).then_inc(sem, 16)

            @block.sync
            def _(sync):
                # SP queue: Wq3+B (gates first scale+PE burst), Wq2, u q2
                w_dma(sync, 3, WQ + BCOLS, sem_w[3])
                w_dma(sync, 2, WQ, sem_w[2])
                u_dma(sync, 2, sem_u[2])
                sync.wait_ge(sem_fin, 1)
                sync.dma_start(out[:, :], ot[:]).then_inc(sem_out, 16)

            @block.gpsimd
            def _(gpsimd):
                # Pool queue: the three other U quarters, arrival order
                u_dma(gpsimd, 3, sem_u[3])
                u_dma(gpsimd, 1, sem_u[1])
                u_dma(gpsimd, 0, sem_u[0])

            @block.scalar
            def _(scalar):
                # ACT queue: the two late W quarters
                w_dma(scalar, 1, WQ, sem_w[1])
                w_dma(scalar, 0, WQ, sem_w[0])
                # warm the ln/exp ACT table (1.3us load) while DMAs stream
                scalar.activation(out=warm[:, 0:1], in_=one, func=AF.Ln)
                scalar.activation(out=warm[:, 1:2], in_=one, func=AF.Exp)
                # epilogue: ss[b,t] = sum_i S[b,t,i]^2 straight from PSUM
                # (Square is in every ACT table; accum_out does the sum)
                scalar.wait_ge(sem_pe, 1)
                s3a = ps[:].rearrange("b (t i) -> b t i", i=DD)
                for t in range(DC):
                    scalar.activation(
                        out=sq[:, t * DD : (t + 1) * DD],
                        in_=s3a[:, t],
                        func=AF.Square,
                        accum_out=ss[:, t : t + 1],
                    )
                # norm = exp(0.5*ln(ss)); et = exp(-norm); 1/norm on DVE
                scalar.activation(out=lss[:], in_=ss[:], func=AF.Ln)
                scalar.activation(
                    out=nrm[:], in_=lss[:], func=AF.Exp, scale=0.5
                ).then_inc(sem_nm, 1)
                scalar.activation(
                    out=et[:], in_=nrm[:], func=AF.Exp, scale=-1.0
                ).then_inc(sem_act, 1)

            def scale_q(eng, q, sem):
                """(1 + B) * W over W quarter q (chunks q*QC..), fused"""
                lo = q * QC
                w_v = wb[
                    :, _wcol(lo) : _wcol(lo) + QC * DIC
                ].rearrange("p (c t i) -> p c t i", t=DC, i=DD)
                eng.scalar_tensor_tensor(
                    out=w_v,
                    in0=wb[:, OFF_B : OFF_B + BCOLS]
                    .rearrange("p (c t) -> p c t", t=DC)[:, lo : lo + QC]
                    .broadcast_to([P, QC, DC, DD]),
                    scalar=1.0,
                    in1=w_v,
                    op0=ALU.add,
                    op1=ALU.mult,
                ).then_inc(sem, 1)

            @block.vector
            def _(vector):
                # quarter-granular (1+B)*W in PE consumption order: only
                # the first ~500ns scale gates the PE start.  B rides in
                # the Wq3 piece, so sem_w[3] covers every quarter's in0.
                for q in (3, 2, 1, 0):
                    vector.wait_ge(sem_w[q], 16)
                    scale_q(vector, q, sem_sq[q])
                # 1/norm in parallel with ACT's exp(-norm)
                s3 = ps[:].rearrange("b (t i) -> b t i", i=DD)
                vector.wait_ge(sem_nm, 1)
                vector.reciprocal(out=rn[:], in_=nrm[:]).then_inc(sem_rn, 1)
                # finisher: fac' = (et - 1) * rn  ;  out = (-S) * fac'
                # (two fused ops + self-sem: DVE does not interlock RAW)
                vector.wait_ge(sem_rn, 1)
                vector.wait_ge(sem_act, 1)
                vector.scalar_tensor_tensor(
                    out=fac[:],
                    in0=et[:],
                    scalar=1.0,
                    in1=rn[:],
                    op0=ALU.subtract,
                    op1=ALU.mult,
                ).then_inc(sem_fp, 1)
                vector.wait_ge(sem_fp, 1)
                vector.scalar_tensor_tensor(
                    out=ot[:].rearrange("b (t i) -> b t i", i=DD),
                    in0=s3,
                    in1=fac[:].broadcast_to([BLOC, DC, DD]),
                    scalar=-1.0,
                    op0=ALU.mult,
                    op1=ALU.mult,
                ).then_inc(sem_fin, 1)

            @block.tensor
            def _(tensor):
                # single accumulation group over all 32 chunks, in DMA
                # arrival order
                first = True
                mm = None
                for q in (3, 2, 1, 0):
                    tensor.wait_ge(sem_sq[q], 1)
                    tensor.wait_ge(sem_u[q], 16)
                    for c in range(q * QC, (q + 1) * QC):
                        mm = tensor.matmul(
                            ps[:],
                            lhsT=u_all[:, c * BLOC : (c + 1) * BLOC],
                            rhs=wb[:, _wcol(c) : _wcol(c) + DIC],
                            start=first,
                            stop=(q == 0 and c == QC - 1),
                            skip_group_check=True,
                        )
                        first = False
                mm.then_inc(sem_pe, 1)

    return nc


_CACHE = {}


def _get_nc():
    if "nc" not in _CACHE:
        _CACHE["nc"] = build_raw()
    return _CACHE["nc"]


def prep_inputs(primary_caps, W, B):
    """Host-side layout prep + sharding + bf16 cast (no arithmetic).

    Contraction row order: chunk c holds n in [c*16, (c+1)*16); within a
    chunk, partition p = j*16 + n_local.  Core k owns batch half k//4
    (32 b's) and digit group GD[k%4] (zeros in unused t slots).
    """
    U = np.asarray(primary_caps, dtype=np.float32)
    Wf = np.asarray(W, dtype=np.float32)
    Bf = np.asarray(B, dtype=np.float32).reshape(D, N)

    # U^T per batch half: [p, (c b_local)]
    Unj = np.transpose(U, (1, 2, 0))  # n j b
    Upcb = (
        Unj.reshape(NCHUNK, 16, DP, BFULL)
        .transpose(0, 2, 1, 3)
        .reshape(NCHUNK, P, BFULL)
        .transpose(1, 0, 2)               # p c b
    )
    Ut = [
        np.ascontiguousarray(
            Upcb[:, :, h * BLOC : (h + 1) * BLOC].reshape(P, UCOLS)
        ).astype(ml_dtypes.bfloat16)
        for h in range(2)
    ]

    # W rows [p, c, d, i] and B rows [d, c, n_l]
    Wnj = np.transpose(Wf, (1, 3, 0, 2))  # n j d i
    Wc = (
        Wnj.reshape(NCHUNK, 16, DP, D, DD)
        .transpose(0, 2, 1, 3, 4)          # c j n_l d i
        .reshape(NCHUNK, P, D, DD)
        .transpose(1, 0, 2, 3)             # p c d i
    )
    Bn = Bf.reshape(D, NCHUNK, 16)         # d c n_l

    wbs = []
    for g in range(4):
        wt = np.zeros((P, NCHUNK, DC, DD), dtype=np.float32)
        bpt = np.zeros((16, NCHUNK, DC), dtype=np.float32)
        for t, d in enumerate(GD[g]):
            wt[:, :, t, :] = Wc[:, :, d, :]
            bpt[:, :, t] = Bn[d].T
        bpm = np.broadcast_to(
            bpt.reshape(1, 16, BCOLS), (DP, 16, BCOLS)
        ).reshape(P, BCOLS)
        wbs.append(
            np.ascontiguousarray(
                np.concatenate(
                    [
                        wt[:, 3 * QC :].reshape(P, WQ),        # Wq3
                        bpm,                                   # B
                        wt[:, 2 * QC : 3 * QC].reshape(P, WQ), # Wq2
                        wt[:, QC : 2 * QC].reshape(P, WQ),     # Wq1
                        wt[:, :QC].reshape(P, WQ),             # Wq0
                    ],
                    axis=1,
                )
            ).astype(ml_dtypes.bfloat16)
        )

    return [
        {"u_t": Ut[k // 4], "wb_t": wbs[k % 4]} for k in range(NCORES)
    ]


def kernel(primary_caps, W, B):
    nc = _get_nc()
    in_maps = prep_inputs(primary_caps, W, B)
    res = run_bass_kernel_spmd(nc, in_maps, core_ids=list(range(NCORES)))
    full = np.empty((BFULL, D, DD), dtype=np.float32)
    for k in range(NCORES):
        h, g = k // 4, k % 4
        o = np.asarray(res.results[k]["out"]).reshape(BLOC, DC, DD)
        for t, d in enumerate(GD[g]):
            full[h * BLOC : (h + 1) * BLOC, d, :] = o[:, t, :]
    return full


# revision 49
# speedup vs baseline: 1.0407x; 1.0407x over previous
"""Trainium2 Bass kernel for nn_DigitCap (sparse_attention).

Math note: the reference's softmax is over a size-1 axis, so C == 1 exactly
and the N x N attention matrix A is dead code.  The computation collapses to

    S[b,d,i]  = sum_{n,j} (1 + B[d,n]) * W[d,n,i,j] * U[b,n,j]
    out[b,d,:] = (1 - exp(-|S|)) * S / (|S| + 1e-7)

Sharding: 2 batch halves x 4 digit groups {0,1,2},{3,4,5},{6,7,8},{9,-,-}
over the 8 cores (core k: batch half k//4, digit group k%4).  This is the
byte-optimal grid: per core 256KB of U + 384KB of W (bf16) vs 512+256 for
pure digit sharding -- the measured DMA fabric tops out ~190GB/s aggregate
(~95GB/s per queue), so bytes on the heaviest queue are the whole game.

Key implementation notes:
  * bf16 inputs (tolerance is 2e-2; bf16 keeps us ~2.5e-3): halves HBM
    traffic and runs the PE at 1 cycle/row instead of fp32's 4.
  * 6 input DMAs balanced over the SP/ACT/Pool queues, >=1KB per-partition
    runs, ordered so each piece's ~900ns completion-semaphore lands just
    before the PE needs it; each dma_start also costs ~600ns of sequencer.
  * single PSUM accumulation group across all 32 matmuls, consumption
    order [24-31, 16-23, 8-15, 0-7] tracking DMA arrival.
  * (1+B)*W fused scale runs per W-quarter on DVE so only a ~350ns scale
    gates the first PE burst.
  * epilogue needs sqrt AND exp, which live in different ACT tables
    (1283ns load each).  sqrt(x) = exp(0.5*ln(x)) lets the whole epilogue
    use the one natural_log+exp table, pre-warmed during the DMA phase;
    1/norm comes from DVE reciprocal(norm) in parallel with ACT's exp.
    ln(0) = -inf on the pad capsules -> NaN outputs there; host discards.
  * DVE does NOT interlock same-engine read-after-write (8-deep exec
    queue reads stale operands), so the finisher is two fused
    scalar_tensor_tensor ops with one self-semaphore between them:
        fac' = (et - 1) * rn ; out = (-S) * fac'
"""

import numpy as np
import ml_dtypes
from contextlib import ExitStack

import concourse.bass as bass
import concourse.mybir as mybir
from concourse.bass_utils import run_bass_kernel_spmd

F32 = mybir.dt.float32
BF16 = mybir.dt.bfloat16
AF = mybir.ActivationFunctionType
ALU = mybir.AluOpType
P = 128
D, DD, N, DP = 10, 512 // 64 + 8, 512, 8  # placeholder; fixed below
D, DD, N, DP = 10, 16, 512, 8     # digit caps, digit dim, primary caps, primary dim
K = N * DP                         # 4096 contraction
NCHUNK = K // P                    # 32 chunks of 128 contraction rows
NCORES = 8
BFULL = 64
GD = [[0, 1, 2], [3, 4, 5], [6, 7, 8], [9]]   # digit groups (t-slots)
DC = 3                             # d slots per core (pads where group < 3)
BLOC = 32                          # batch rows per core (half of 64)
DIC = DC * DD                      # 48 output cols per core
HC = NCHUNK // 2                   # 16
QC = NCHUNK // 4                   # 8 chunks per quarter
WHALF = HC * DIC                   # 768 cols per W half
BCOLS = NCHUNK * DC                # 96
UCOLS = NCHUNK * BLOC              # 1024
# wb DRAM/SBUF layout: [Wq3 | B | Wq2 | Wq1 | Wq0], consumption order,
# so the first DMA piece (Wq3+B, 120KB) lands ~1.2us before a half would
WQ = QC * DIC                      # 384 cols per W quarter
WB_COLS = 4 * WQ + BCOLS           # 1632
OFF_B = WQ                         # 384
OFF_WQ = {3: 0, 2: WQ + BCOLS, 1: 2 * WQ + BCOLS, 0: 3 * WQ + BCOLS}


def _wcol(c):
    """start col of chunk c's W block inside wb"""
    q = c // QC
    return OFF_WQ[q] + (c - q * QC) * DIC


def build_raw():
    nc = bass.Bass()
    u_t = nc.dram_tensor("u_t", [P, UCOLS], BF16, kind="ExternalInput")
    wb_t = nc.dram_tensor("wb_t", [P, WB_COLS], BF16, kind="ExternalInput")
    out = nc.dram_tensor("out", [BLOC, DIC], F32, kind="ExternalOutput")

    with ExitStack() as ctx:
        u_all = ctx.enter_context(nc.sbuf_tensor("u_all", [P, UCOLS], BF16))
        wb = ctx.enter_context(nc.sbuf_tensor("wb", [P, WB_COLS], BF16))
        ps = ctx.enter_context(nc.psum_tensor("ps", [BLOC, DIC], F32))
        sq = ctx.enter_context(nc.sbuf_tensor("sq", [BLOC, DIC], F32))
        ss = ctx.enter_context(nc.sbuf_tensor("ss", [BLOC, DC], F32))
        lss = ctx.enter_context(nc.sbuf_tensor("lss", [BLOC, DC], F32))
        nrm = ctx.enter_context(nc.sbuf_tensor("nrm", [BLOC, DC], F32))
        rn = ctx.enter_context(nc.sbuf_tensor("rn", [BLOC, DC], F32))
        et = ctx.enter_context(nc.sbuf_tensor("et", [BLOC, DC], F32))
        fac = ctx.enter_context(nc.sbuf_tensor("fac", [BLOC, DC], F32))
        ot = ctx.enter_context(nc.sbuf_tensor("ot", [BLOC, DIC], F32))
        warm = ctx.enter_context(nc.sbuf_tensor("warm", [1, 2], F32))
        sem_w = [ctx.enter_context(nc.semaphore(f"sem_w{q}")) for q in range(4)]
        # u piece sems, indexed by quarter q (chunks q*QC..(q+1)*QC)
        sem_u = [ctx.enter_context(nc.semaphore(f"sem_u{q}")) for q in range(4)]
        sem_sq = [ctx.enter_context(nc.semaphore(f"sem_sq{q}")) for q in range(4)]
        sem_pe = ctx.enter_context(nc.semaphore("sem_pe"))
        sem_nm = ctx.enter_context(nc.semaphore("sem_nm"))
        sem_rn = ctx.enter_context(nc.semaphore("sem_rn"))
        sem_act = ctx.enter_context(nc.semaphore("sem_act"))
        sem_fp = ctx.enter_context(nc.semaphore("sem_fp"))
        sem_fin = ctx.enter_context(nc.semaphore("sem_fin"))
        sem_out = ctx.enter_context(nc.semaphore("sem_out"))

        one = nc.const_aps.tensor(1.0, (1, 1), F32)

        def u_dma(eng, q, sem):
            eng.dma_start(
                u_all[:, q * QC * BLOC : (q + 1) * QC * BLOC],
                bass.AP(u_t, q * QC * BLOC, [[UCOLS, P], [1, QC * BLOC]]),
            ).then_inc(sem, 16)

        with nc.Block() as block:

            def w_dma(eng, q, ncols, sem):
                off = OFF_WQ[q]
                eng.dma_start(
                    wb[:, off : off + ncols],
                    bass.AP(wb_t, off, [[WB_COLS, P], [1, ncols]]),
                ).then_inc(sem, 16)

            @block.sync
            def _(sync):
                # consecutive-consumed pieces ride DIFFERENT queues: PE
                # eats a quarter in ~0.5us but each queue delivers one
                # only every ~1.1us
                w_dma(sync, 3, WQ + BCOLS, sem_w[3])
                w_dma(sync, 1, WQ, sem_w[1])
                u_dma(sync, 0, sem_u[0])
                sync.wait_ge(sem_fin, 1)
                sync.dma_start(out[:, :], ot[:]).then_inc(sem_out, 16)

            @block.gpsimd
            def _(gpsimd):
                u_dma(gpsimd, 3, sem_u[3])
                u_dma(gpsimd, 2, sem_u[2])
                u_dma(gpsimd, 1, sem_u[1])

            @block.scalar
            def _(scalar):
                w_dma(scalar, 2, WQ, sem_w[2])
                w_dma(scalar, 0, WQ, sem_w[0])
                # warm the ln/exp ACT table (1.3us load) while DMAs stream
                scalar.activation(out=warm[:, 0:1], in_=one, func=AF.Ln)
                scalar.activation(out=warm[:, 1:2], in_=one, func=AF.Exp)
                # epilogue: ss[b,t] = sum_i S[b,t,i]^2 straight from PSUM
                # (Square is in every ACT table; accum_out does the sum)
                scalar.wait_ge(sem_pe, 1)
                s3a = ps[:].rearrange("b (t i) -> b t i", i=DD)
                for t in range(DC):
                    scalar.activation(
                        out=sq[:, t * DD : (t + 1) * DD],
                        in_=s3a[:, t],
                        func=AF.Square,
                        accum_out=ss[:, t : t + 1],
                    )
                # norm = exp(0.5*ln(ss)); et = exp(-norm); 1/norm on DVE
                scalar.activation(out=lss[:], in_=ss[:], func=AF.Ln)
                scalar.activation(
                    out=nrm[:], in_=lss[:], func=AF.Exp, scale=0.5
                ).then_inc(sem_nm, 1)
                scalar.activation(
                    out=et[:], in_=nrm[:], func=AF.Exp, scale=-1.0
                ).then_inc(sem_act, 1)

            def scale_q(eng, q, sem):
                """(1 + B) * W over W quarter q (chunks q*QC..), fused"""
                lo = q * QC
                w_v = wb[
                    :, _wcol(lo) : _wcol(lo) + QC * DIC
                ].rearrange("p (c t i) -> p c t i", t=DC, i=DD)
                eng.scalar_tensor_tensor(
                    out=w_v,
                    in0=wb[:, OFF_B : OFF_B + BCOLS]
                    .rearrange("p (c t) -> p c t", t=DC)[:, lo : lo + QC]
                    .broadcast_to([P, QC, DC, DD]),
                    scalar=1.0,
                    in1=w_v,
                    op0=ALU.add,
                    op1=ALU.mult,
                ).then_inc(sem, 1)

            @block.vector
            def _(vector):
                # quarter-granular (1+B)*W in PE consumption order: only
                # the first ~500ns scale gates the PE start.  B rides in
                # the Wq3 piece, so sem_w[3] covers every quarter's in0.
                for q in (3, 2, 1, 0):
                    vector.wait_ge(sem_w[q], 16)
                    scale_q(vector, q, sem_sq[q])
                # 1/norm in parallel with ACT's exp(-norm)
                s3 = ps[:].rearrange("b (t i) -> b t i", i=DD)
                vector.wait_ge(sem_nm, 1)
                vector.reciprocal(out=rn[:], in_=nrm[:]).then_inc(sem_rn, 1)
                # finisher: fac' = (et - 1) * rn  ;  out = (-S) * fac'
                # (two fused ops + self-sem: DVE does not interlock RAW)
                vector.wait_ge(sem_rn, 1)
                vector.wait_ge(sem_act, 1)
                vector.scalar_tensor_tensor(
                    out=fac[:],
                    in0=et[:],
                    scalar=1.0,
                    in1=rn[:],
                    op0=ALU.subtract,
                    op1=ALU.mult,
                ).then_inc(sem_fp, 1)
                vector.wait_ge(sem_fp, 1)
                vector.scalar_tensor_tensor(
                    out=ot[:].rearrange("b (t i) -> b t i", i=DD),
                    in0=s3,
                    in1=fac[:].broadcast_to([BLOC, DC, DD]),
                    scalar=-1.0,
                    op0=ALU.mult,
                    op1=ALU.mult,
                ).then_inc(sem_fin, 1)

            @block.tensor
            def _(tensor):
                # single accumulation group over all 32 chunks, in DMA
                # arrival order
                first = True
                mm = None
                for q in (3, 2, 1, 0):
                    tensor.wait_ge(sem_sq[q], 1)
                    tensor.wait_ge(sem_u[q], 16)
                    for c in range(q * QC, (q + 1) * QC):
                        mm = tensor.matmul(
                            ps[:],
                            lhsT=u_all[:, c * BLOC : (c + 1) * BLOC],
                            rhs=wb[:, _wcol(c) : _wcol(c) + DIC],
                            start=first,
                            stop=(q == 0 and c == QC - 1),
                            skip_group_check=True,
                        )
                        first = False
                mm.then_inc(sem_pe, 1)

    return nc


_CACHE = {}


def _get_nc():
    if "nc" not in _CACHE:
        _CACHE["nc"] = build_raw()
    return _CACHE["nc"]


def prep_inputs(primary_caps, W, B):
    """Host-side layout prep + sharding + bf16 cast (no arithmetic).

    Contraction row order: chunk c holds n in [c*16, (c+1)*16); within a
    chunk, partition p = j*16 + n_local.  Core k owns batch half k//4
    (32 b's) and digit group GD[k%4] (zeros in unused t slots).
    """
    U = np.asarray(primary_caps, dtype=np.float32)
    Wf = np.asarray(W, dtype=np.float32)
    Bf = np.asarray(B, dtype=np.float32).reshape(D, N)

    # U^T per batch half: [p, (c b_local)]
    Unj = np.transpose(U, (1, 2, 0))  # n j b
    Upcb = (
        Unj.reshape(NCHUNK, 16, DP, BFULL)
        .transpose(0, 2, 1, 3)
        .reshape(NCHUNK, P, BFULL)
        .transpose(1, 0, 2)               # p c b
    )
    Ut = [
        np.ascontiguousarray(
            Upcb[:, :, h * BLOC : (h + 1) * BLOC].reshape(P, UCOLS)
        ).astype(ml_dtypes.bfloat16)
        for h in range(2)
    ]

    # W rows [p, c, d, i] and B rows [d, c, n_l]
    Wnj = np.transpose(Wf, (1, 3, 0, 2))  # n j d i
    Wc = (
        Wnj.reshape(NCHUNK, 16, DP, D, DD)
        .transpose(0, 2, 1, 3, 4)          # c j n_l d i
        .reshape(NCHUNK, P, D, DD)
        .transpose(1, 0, 2, 3)             # p c d i
    )
    Bn = Bf.reshape(D, NCHUNK, 16)         # d c n_l

    wbs = []
    for g in range(4):
        wt = np.zeros((P, NCHUNK, DC, DD), dtype=np.float32)
        bpt = np.zeros((16, NCHUNK, DC), dtype=np.float32)
        for t, d in enumerate(GD[g]):
            wt[:, :, t, :] = Wc[:, :, d, :]
            bpt[:, :, t] = Bn[d].T
        bpm = np.broadcast_to(
            bpt.reshape(1, 16, BCOLS), (DP, 16, BCOLS)
        ).reshape(P, BCOLS)
        wbs.append(
            np.ascontiguousarray(
                np.concatenate(
                    [
                        wt[:, 3 * QC :].reshape(P, WQ),        # Wq3
                        bpm,                                   # B
                        wt[:, 2 * QC : 3 * QC].reshape(P, WQ), # Wq2
                        wt[:, QC : 2 * QC].reshape(P, WQ),     # Wq1
                        wt[:, :QC].reshape(P, WQ),             # Wq0
                    ],
                    axis=1,
                )
            ).astype(ml_dtypes.bfloat16)
        )

    return [
        {"u_t": Ut[k // 4], "wb_t": wbs[k % 4]} for k in range(NCORES)
    ]


def kernel(primary_caps, W, B):
    nc = _get_nc()
    in_maps = prep_inputs(primary_caps, W, B)
    res = run_bass_kernel_spmd(nc, in_maps, core_ids=list(range(NCORES)))
    full = np.empty((BFULL, D, DD), dtype=np.float32)
    for k in range(NCORES):
        h, g = k // 4, k % 4
        o = np.asarray(res.results[k]["out"]).reshape(BLOC, DC, DD)
        for t, d in enumerate(GD[g]):
            full[h * BLOC : (h + 1) * BLOC, d, :] = o[:, t, :]
    return full
